# revision 30
# baseline (speedup 1.0000x reference)
"""TRN2 Bass kernel for multi-head self-attention with RoPE (causal).

Problem: B=4, S=2048, D=768, H=12 heads of dk=64, fp32 in/out.

Sharding: 8 cores = 4 batches x 2 head-groups of 6 heads. Each core computes
QKV projections for its 6 heads, RoPE, causal flash attention, and a
partial output projection; the host sums the two partials per batch.

Numerics: split-bf16 (hi+lo) 3-term matmuls for the Q/K projections and
for Q.K^T scores (the softmax is hyper-argmax: scaled score std ~600,
top-2 gap ~150, so bf16/tf32 rounding would flip winners). Row max must
be EXACT: subsampled estimates overflow bf16 p (gap spacing ~150 >> the
88-e-fold bf16 window). V/AV/O-proj in plain bf16.

Structure (v3, PE-density rewrite):
  - V projection upfront (dense warm-up stream) into v_ext with an
    appended ones column per head, so the AV matmul's 65th output row is
    the softmax denominator for free.
  - Main loop: per pair p, per 512-col x chunk c: Q/K proj matmuls + rope
    (spread over DVE/GpSimd/ACT/DMA), interleaved with attention S/A
    groups of already-projected heads so the PE never idles and DVE's
    attention-phase overhang (exact row max) is absorbed into the proj
    windows.
  - S-group (h, g=4 q-tiles): scores (banded 2-term matmul + k_lo
    correction), mask, exact row max, exp on ACT, XBAR transpose to pts.
  - A-group: column-major AV (one accumulation group per (h,g), k-block
    matmuls of width <=512), flash merge = single per-column rescale of
    the AV psum at the k=1024 boundary, then normalize via
    reciprocal_approx_fast + partition_broadcast into av_all.
  - O-projection tail.
"""

import sys

sys.path.insert(0, "/opt/trn_rl_repo")

from contextlib import ExitStack

import ml_dtypes
import numpy as np

import concourse.bass as bass
import concourse.tile as tile
from concourse import bacc, mybir
from concourse.bass_utils import run_bass_kernel_spmd

F32 = mybir.dt.float32
BF16 = mybir.dt.bfloat16
bf16 = ml_dtypes.bfloat16

B, D, H, DK = 4, 768, 12, 64
NHC = 6          # heads per core
NPAIR = 3        # head pairs per core
DSUB = 6         # d_in subtiles of 128
CPC = NHC * DK   # 384 head-dims per core
CH = 1024        # score chunk along k


def _build(S=2048, trace_label="", debug_out=False):
    NQT = S // 128       # 16 q-tiles
    NG = NQT // 4        # 4 q-groups per head (512 q cols each)
    nc = bacc.Bacc("TRN2", target_bir_lowering=False, debug=False, num_devices=8)

    def din(name, shape, dt):
        return nc.dram_tensor(name, shape, dt, kind="ExternalInput").ap()

    xh_d = din("xh", [128, DSUB, S], BF16)
    xl_d = din("xl", [128, DSUB, S], BF16)
    wqh_d = din("wqh", [128, DSUB, CPC], BF16)
    wql_d = din("wql", [128, DSUB, CPC], BF16)
    wkh_d = din("wkh", [128, DSUB, CPC], BF16)
    wkl_d = din("wkl", [128, DSUB, CPC], BF16)
    wv_d = din("wvT", [128, DSUB, CPC], BF16)
    wo_d = din("woT", [128, NPAIR, D], BF16)
    cos_d = din("cos_t", [128, S], F32)
    sin_d = din("sin_t", [128, S], F32)
    mask_d = din("mask", [128, 128], F32)
    out_d = nc.dram_tensor("out", [S, D], F32, kind="ExternalOutput").ap()
    if debug_out:
        rec_d = nc.dram_tensor("rec_dbg", [NHC, S], F32,
                               kind="ExternalOutput").ap()
        avd_d = nc.dram_tensor("av_dbg", [128, NPAIR, S], BF16,
                               kind="ExternalOutput").ap()
        q_d = nc.dram_tensor("q_dbg", [128, NHC, S], BF16,
                             kind="ExternalOutput").ap()
        k_d = nc.dram_tensor("k_dbg", [128, NHC, S], BF16,
                             kind="ExternalOutput").ap()
        psb_d = nc.dram_tensor("psb_dbg", [4, 128, CH], BF16,
                               kind="ExternalOutput").ap()
        pts_d = nc.dram_tensor("pts_dbg", [128, 4, NQT, 128], BF16,
                               kind="ExternalOutput").ap()
        avp_d = nc.dram_tensor("avp_dbg", [65, 512], F32,
                               kind="ExternalOutput").ap()

    SUB = mybir.AluOpType.subtract
    MIN = mybir.AluOpType.min

    with tile.TileContext(nc) as tc, ExitStack() as ctx:
        # ---------- persistent SBUF ----------
        pers = ctx.enter_context(tc.tile_pool(name="pers", bufs=1))

        def load(pool, dr, name):
            t = pool.tile(list(dr.shape), dr.dtype, tag=f"L{name}")
            nc.sync.dma_start(t[:], dr[:])
            return t

        mask = load(pers, mask_d, "mask")
        cos_t = load(pers, cos_d, "cos")
        sin_t = load(pers, sin_d, "sin")

        # band layouts (all matmul operands at base partition 0):
        # q_hl: band0 = q_hi, band1 = q_lo; k_hh: k_hi in both bands;
        # k_l: k_lo on partitions 0:64
        q_hl = pers.tile([128, NHC, S], BF16, tag="q_hl")
        k_hh = pers.tile([128, NHC, S], BF16, tag="k_hh")
        k_l = pers.tile([64, NHC, S], BF16, tag="k_l")
        # v with an appended ones column per head: AV matmul row 64 = sum(p)
        v_ext = pers.tile([128, NQT, NHC, 65], BF16, tag="v_ext")
        av_all = pers.tile([128, NPAIR, S], BF16, tag="av_all")

        # ---------- V projection upfront ----------
        with tc.tile_pool(name="vw", bufs=1) as vw, \
             tc.tile_pool(name="vx", bufs=2) as vx, \
             tc.tile_pool(name="pv", bufs=2, space="PSUM") as pvs:
            wv = load(vw, wv_d, "wv")
            nc.gpsimd.memset(v_ext[:, :, :, 64:65], 1.0)
            for sc_i in range(S // 512):
                ssl = bass.ts(sc_i, 512)
                xv = vx.tile([128, DSUB, 512], BF16, tag="xv")
                nc.sync.dma_start(xv[:], xh_d[:, :, ssl])
                for st4 in range(4):
                    st = sc_i * 4 + st4
                    psv = pvs.tile([128, 512], F32, tag="pv")
                    for t in range(DSUB):
                        nc.tensor.matmul(
                            psv[:, 0:CPC],
                            xv[:, t, bass.ts(st4, 128)], wv[:, t, :],
                            start=(t == 0), stop=(t == DSUB - 1),
                        )
                    nc.scalar.copy(out=v_ext[:, st, :, 0:64], in_=psv[:, 0:CPC])

        # ---------- main: Q/K proj pairs interleaved with attention ----------
        with tc.tile_pool(name="wsl", bufs=2) as wsl, \
             tc.tile_pool(name="bx", bufs=2) as bx, \
             tc.tile_pool(name="rwork", bufs=2) as rwk, \
             tc.tile_pool(name="pqk", bufs=1, space="PSUM") as pps, \
             tc.tile_pool(name="scps", bufs=2, space="PSUM") as scps, \
             tc.tile_pool(name="avps", bufs=2, space="PSUM") as avps, \
             tc.tile_pool(name="psbp", bufs=2) as psbp, \
             tc.tile_pool(name="ptsp", bufs=2) as ptsp, \
             tc.tile_pool(name="stats", bufs=4) as stp, \
             tc.tile_pool(name="gdp", bufs=2) as gdp, \
             tc.tile_pool(name="b0p", bufs=2) as b0p, \
             tc.tile_pool(name="nwork", bufs=1) as nwk:

            def load_wslices(p):
                ws = {}
                for nm, dr in (("qh", wqh_d), ("ql", wql_d),
                               ("kh", wkh_d), ("kl", wkl_d)):
                    t = wsl.tile([128, DSUB, 128], BF16, tag=f"w{nm}")
                    nc.sync.dma_start(t[:], dr[:, :, bass.ts(p, 128)])
                    ws[nm] = t
                return ws

            def proj_chunk(p, c, ws):
                ssl = bass.ts(c, 512)
                xc = bx.tile([128, 2, DSUB, 512], BF16, tag="xc")
                nc.sync.dma_start(xc[:, 0], xh_d[:, :, ssl])
                nc.sync.dma_start(xc[:, 1], xl_d[:, :, ssl])
                pqk = pps.tile([128, 2, 512], F32, tag="pp")
                for qk, (w_hi, w_lo) in enumerate(
                    ((ws["qh"], ws["ql"]), (ws["kh"], ws["kl"]))
                ):
                    n = 0
                    for t in range(DSUB):
                        for lh, xi in ((w_hi, 0), (w_hi, 1), (w_lo, 0)):
                            nc.tensor.matmul(
                                pqk[:, qk, :],
                                lh[:, t, :],
                                xc[:, xi, t, :],
                                start=(n == 0), stop=(n == 3 * DSUB - 1),
                            )
                            n += 1
                # rope (2 heads stacked on partitions)
                for qk in range(2):
                    f32c = rwk.tile([128, 512], F32, tag="f32c")
                    nc.scalar.copy(out=f32c[:], in_=pqk[:, qk, :])
                    swp = rwk.tile([128, 512], F32, tag="swp")
                    for a in range(2):
                        nc.sync.dma_start(
                            swp[64 * a:64 * a + 32, :],
                            f32c[64 * a + 32:64 * a + 64, :],
                        )
                        nc.sync.dma_start(
                            swp[64 * a + 32:64 * a + 64, :],
                            f32c[64 * a:64 * a + 32, :],
                        )
                    # rotate in place: f32c *= cos (after swaps read it),
                    # swp = swp*sin + f32c
                    nc.vector.tensor_mul(f32c[:], f32c[:], cos_t[:, ssl])
                    nc.gpsimd.tensor_mul(swp[:], swp[:], sin_t[:, ssl])
                    nc.gpsimd.tensor_add(swp[:], swp[:], f32c[:])
                    for sub in range(2):
                        hh = 2 * p + sub
                        band = swp[64 * sub:64 * sub + 64, :]
                        if sub == 0:
                            b0 = band
                        else:
                            b0t = b0p.tile([64, 512], F32, tag="b0t")
                            nc.sync.dma_start(b0t[:], band)
                            b0 = b0t[:]
                        if qk == 0:
                            nc.scalar.copy(
                                out=q_hl[0:64, hh, ssl], in_=b0)
                            nc.vector.tensor_tensor(
                                q_hl[64:128, hh, ssl], b0,
                                q_hl[0:64, hh, ssl], SUB,
                            )
                        else:
                            nc.scalar.copy(
                                out=k_hh[0:64, hh, ssl], in_=b0)
                            nc.gpsimd.tensor_tensor(
                                k_l[0:64, hh, ssl], b0,
                                k_hh[0:64, hh, ssl], SUB,
                            )
                            nc.sync.dma_start(
                                k_hh[64:128, hh, ssl],
                                k_hh[0:64, hh, ssl],
                            )

            pts_tiles = {}
            gd_tiles = {}

            def s_group(gi, h, g):
                # pts[p, qtl, j, c]: p^T of q-tile (4g+qtl), k-block j;
                # transpose DMA dst [:, qtl, j0:j0+nblk, :] is contiguous
                pts = ptsp.tile([128, 4, NQT, 128], BF16, tag="pts")
                pts_tiles[gi] = pts
                if g >= 2:
                    gd = gdp.tile([1, 512], BF16, tag="gdelta")
                    gd_tiles[gi] = gd
                for qtl in range(4):
                    qt = 4 * g + qtl
                    nk = (qt + 1) * 128
                    qsl = bass.ts(qt, 128)
                    chunks = [(0, min(CH, nk))]
                    if nk > CH:
                        chunks.append((CH, nk - CH))
                    nm1 = None
                    for ci, (k0, nkc) in enumerate(chunks):
                        sc = scps.tile([128, CH], F32, tag="sc")
                        for n0 in range(0, nkc, 512):
                            nn = min(512, nkc - n0)
                            ksl = bass.ds(k0 + n0, nn)
                            nc.tensor.matmul(
                                sc[:, bass.ds(n0, nn)],
                                q_hl[:, h, qsl], k_hh[:, h, ksl],
                                start=True, stop=False,
                            )
                            nc.tensor.matmul(
                                sc[:, bass.ds(n0, nn)],
                                q_hl[0:64, h, qsl], k_l[:, h, ksl],
                                start=False, stop=True,
                            )
                        if ci == len(chunks) - 1:
                            doff = nk - 128 - k0
                            nc.vector.tensor_add(
                                sc[:, bass.ds(doff, 128)],
                                sc[:, bass.ds(doff, 128)], mask[:],
                            )
                        mx = stp.tile([128, 1], F32, tag="mx")
                        nc.vector.tensor_reduce(
                            mx[:], sc[:, 0:nkc], mybir.AxisListType.X,
                            mybir.AluOpType.max,
                        )
                        nm8 = stp.tile([128, 1], F32, tag=f"nm{ci}")
                        nc.vector.tensor_scalar_mul(nm8[:], mx[:], -0.125)
                        if ci == 0:
                            nm1 = nm8
                            bias = nm8
                        else:
                            nmf = stp.tile([128, 1], F32, tag="nmf")
                            nc.vector.tensor_tensor(
                                nmf[:], nm8[:], nm1[:], MIN)
                            delta = stp.tile([128, 1], BF16, tag="delta")
                            nc.vector.tensor_tensor(
                                delta[:], nmf[:], nm1[:], SUB)
                            nc.sync.dma_start(
                                gd_tiles[gi][0:1, bass.ts(qtl, 128)],
                                delta[:, 0:1],
                            )
                            bias = nmf
                        psb = psbp.tile([128, CH], BF16, tag="psb")
                        nc.scalar.activation(
                            psb[:, 0:nkc], sc[:, 0:nkc],
                            mybir.ActivationFunctionType.Exp,
                            bias=bias[:, 0:1], scale=0.125,
                        )
                        j0 = k0 // 128
                        nblk = nkc // 128
                        nc.sync.dma_start_transpose(
                            pts[:, qtl, j0:j0 + nblk, :],
                            psb[:, 0:nkc],
                        )
                        if debug_out and h == 0 and g == 0:
                            nc.sync.dma_start(
                                psb_d[qtl, :, 0:nkc], psb[:, 0:nkc])

            def a_group(gi, h, g):
                pts = pts_tiles.pop(gi)
                av = avps.tile([65, 512], F32, tag="av")
                jmax = 4 * g + 3
                ph1 = list(range(min(8, jmax + 1)))
                ph2 = list(range(8, jmax + 1))

                def emit(js, first_starts, skip):
                    for idx, j in enumerate(js):
                        q0 = max(j - 4 * g, 0)
                        nc.tensor.matmul(
                            av[:, bass.ds(q0 * 128, (4 - q0) * 128)],
                            v_ext[:, j, h, :],
                            pts[:, q0:4, j, :],
                            start=(first_starts and idx == 0),
                            stop=(idx == len(js) - 1),
                            skip_group_check=skip,
                        )

                emit(ph1, True, False)
                if debug_out and h == 0 and g == 0:
                    nc.sync.dma_start(pts_d[:], pts[:])
                    avc = nwk.tile([65, 512], F32, tag="avc_dbg")
                    nc.scalar.copy(out=avc[:], in_=av[:, :])
                    nc.sync.dma_start(avp_d[:], avc[:])
                if ph2:
                    abr = nwk.tile([1, 512], BF16, tag="abr")
                    nc.scalar.activation(
                        abr[:], gd_tiles.pop(gi)[:],
                        mybir.ActivationFunctionType.Exp,
                    )
                    ab = nwk.tile([65, 512], BF16, tag="ab")
                    nc.gpsimd.partition_broadcast(ab[:], abr[0:1, :])
                    nc.vector.tensor_mul(av[:, :], av[:, :], ab[:])
                    emit(ph2, False, True)
                # normalize into av_all (O-proj lhsT layout)
                dro = nwk.tile([1, 512], F32, tag="dro")
                nc.vector.tensor_copy(dro[:], av[64:65, :])
                rec = nwk.tile([1, 512], F32, tag="rec")
                nc.vector.reciprocal_approx_fast(out=rec[:], in_=dro[:])
                if debug_out:
                    nc.sync.dma_start(
                        rec_d[h:h + 1, bass.ts(g, 512)], rec[:])
                recb = nwk.tile([64, 512], F32, tag="recb")
                nc.gpsimd.partition_broadcast(recb[:], rec[0:1, :])
                hl, pr = h % 2, h // 2
                nc.vector.tensor_mul(
                    av_all[64 * hl:64 * hl + 64, pr, bass.ts(g, 512)],
                    av[0:64, :], recb[:],
                )

            # ---- emission schedule: proj windows absorb attention ----
            sorder = [(2 * p + i, g)
                      for p in range(NPAIR) for g in range(NG) for i in (0, 1)]
            si = 0

            def emit_sa(budget, p, c):
                nonlocal si
                while si < len(sorder) and budget > 0:
                    h, g = sorder[si]
                    if p is not None and not (
                        h // 2 < p or (h // 2 == p and g < c)
                    ):
                        return
                    s_group(si, h, g)
                    if si > 0:
                        a_group(si - 1, *sorder[si - 1])
                    si += 1
                    budget -= 1

            ws_cur = load_wslices(0)
            for p in range(NPAIR):
                ws_next = load_wslices(p + 1) if p < NPAIR - 1 else None
                for c in range(S // 512):
                    proj_chunk(p, c, ws_cur)
                    emit_sa(2, p, c)
                ws_cur = ws_next
            emit_sa(len(sorder), None, None)
            a_group(len(sorder) - 1, *sorder[-1])
            if debug_out:
                nc.sync.dma_start(avd_d[:], av_all[:])
                nc.sync.dma_start(q_d[:], q_hl[:])
                nc.sync.dma_start(k_d[:], k_hh[:])

        # ---------- output projection ----------
        with tc.tile_pool(name="ops", bufs=2, space="PSUM") as ops, \
             tc.tile_pool(name="wop", bufs=1) as wop, \
             tc.tile_pool(name="owork", bufs=3) as owk:
            wo = load(wop, wo_d, "wo")
            for st in range(NQT):
                po = ops.tile([128, 2, 512], F32, tag="po")
                for half in range(2):
                    for p in range(NPAIR):
                        nc.tensor.matmul(
                            po[:, half, 0:384],
                            av_all[:, p, bass.ts(st, 128)],
                            wo[:, p, bass.ts(half, 384)],
                            start=(p == 0), stop=(p == NPAIR - 1),
                        )
                osb = owk.tile([128, D], F32, tag="osb")
                nc.scalar.copy(out=osb[:, 0:384], in_=po[:, 0, 0:384])
                nc.scalar.copy(out=osb[:, 384:768], in_=po[:, 1, 0:384])
                nc.sync.dma_start(out_d[bass.ts(st, 128), :], osb[:])

    nc.compile()
    return nc


def _rope_perm():
    p = np.zeros(DK, dtype=np.int64)
    for i in range(DK // 2):
        p[i] = 2 * i
        p[i + 32] = 2 * i + 1
    return p


def _split(a):
    hi = a.astype(bf16)
    lo = (a.astype(np.float32) - hi.astype(np.float32)).astype(bf16)
    return hi, lo


def _tile_din(a):
    # [768, F] -> [128, 6, F]
    return np.ascontiguousarray(a.reshape(DSUB, 128, -1).transpose(1, 0, 2))


def make_inputs(x, wq, wk, wv, wo, S):
    """Host-side prep: returns list of 8 in_maps (core = 2*b + g)."""
    perm = _rope_perm()
    pos = np.arange(S, dtype=np.float64)
    inv = 10000.0 ** (-2.0 * np.arange(DK // 2, dtype=np.float64) / DK)
    ang = pos[:, None] * inv[None, :]
    cosv = np.cos(ang).astype(np.float32).T  # [32, S]
    sinv = np.sin(ang).astype(np.float32).T
    cos_t = np.tile(cosv, (4, 1)).astype(np.float32)            # [128, S]
    sin_t = np.tile(
        np.concatenate([-sinv, sinv], axis=0), (2, 1)
    ).astype(np.float32)                                        # [128, S]
    mask = np.triu(np.full((128, 128), -1e9, np.float32), 1)

    maps = []
    for b in range(B):
        xT = np.ascontiguousarray(x[b].T.astype(np.float32))  # [768, S]
        xh, xl = _split(xT)
        xh_t, xl_t = _tile_din(xh), _tile_din(xl)
        for g in range(2):
            hs = slice(g * CPC, (g + 1) * CPC)
            wqc = wq[hs].astype(np.float32).copy()
            wkc = wk[hs].astype(np.float32).copy()
            for arr in (wqc, wkc):
                for i in range(NHC):
                    blk = arr[i * DK:(i + 1) * DK].copy()
                    arr[i * DK:(i + 1) * DK] = blk[perm]
            wqh, wql = _split(wqc.T)  # [768, 384]
            wkh, wkl = _split(wkc.T)
            wvT = wv[hs].astype(np.float32).T.astype(bf16)
            woT = wo[:, hs].astype(np.float32).T.astype(bf16)  # [384, 768]
            maps.append({
                "xh": xh_t, "xl": xl_t,
                "wqh": _tile_din(wqh), "wql": _tile_din(wql),
                "wkh": _tile_din(wkh), "wkl": _tile_din(wkl),
                "wvT": _tile_din(wvT),
                "woT": np.ascontiguousarray(
                    woT.reshape(NPAIR, 128, D).transpose(1, 0, 2)),
                "cos_t": cos_t, "sin_t": sin_t, "mask": mask,
            })
    return maps


_PROG = {}


def _prog(S):
    if S not in _PROG:
        _PROG[S] = _build(S)
    return _PROG[S]


def kernel(x, wq, wk, wv, wo, S=2048, trace=False):
    x = np.asarray(x, np.float32)
    nc = _prog(S)
    maps = make_inputs(x, np.asarray(wq), np.asarray(wk), np.asarray(wv),
                       np.asarray(wo), S)
    res = run_bass_kernel_spmd(nc, maps, list(range(8)), trace=trace)
    outs = []
    for b in range(B):
        outs.append(res.results[2 * b]["out"] + res.results[2 * b + 1]["out"])
    out = np.stack(outs)
    if trace:
        kernel.last_exec_time_ns = res.exec_time_ns
        kernel.last_results = res
    return out
